# revision 1
# baseline (speedup 1.0000x reference)
"""Trainium2 Bass kernel for nn_PositionalEncoding_61151744360729.

out[b, s, n, :] = x[b, s, n, :] + ||x[b, s+1, n, :] - x[b, s, n, :]||_2
(with distance 0 at s = S-1).

Sharding: data-parallel on batch across 8 NeuronCores (64 batches/core).
On-core layout: partition p = b*2 + h (b = batch, h = sequence half),
free dim = frames*75 floats, so every DMA is a large contiguous span per
partition and the outermost AP dim (64) lets SWDGE fan descriptors over
all 16 SDMA engines. Each batch is padded host-side with a copy of its
last frame, which makes the last-frame distance exactly 0 with no
special-casing. Per 64-frame chunk: DVE shifted subtract -> ACT square
-> two strided DVE adds (sum over the 3 coords) -> ACT sqrt -> three
strided DVE broadcast-adds -> DMA out.
"""

import sys
from contextlib import ExitStack

for _p in ("/opt/trn_rl_repo", "/root/.axon_site/_ro/trn_rl_repo"):
    if _p not in sys.path:
        sys.path.insert(0, _p)

import numpy as np

import concourse.bass as bass
import concourse.tile as tile
from concourse import bacc, mybir
from concourse.bass_utils import run_bass_kernel_spmd

B, S, N, C = 512, 1024, 25, 3
FW = N * C                 # 75 floats per frame
NCORES = 8
BC = B // NCORES           # 64 batches per core
H = 2                      # sequence halves -> 128 partitions
SH = S // H                # 512 frames per half
P = H * BC                 # 128 partitions
F = 64                     # frames per chunk per partition
K = SH // F                # 8 chunks
IN_FLAT = BC * (S + 1) * FW   # input padded by one zero frame per batch
OUT_FLAT = BC * S * FW

_cache = {}


def _build():
    f32 = mybir.dt.float32
    Af = mybir.ActivationFunctionType
    nc = bacc.Bacc(
        "TRN2", target_bir_lowering=False, debug=False, num_devices=NCORES
    )
    xin = nc.dram_tensor("xin", [IN_FLAT], f32, kind="ExternalInput")
    yout = nc.dram_tensor("yout", [OUT_FLAT], f32, kind="ExternalOutput")

    with tile.TileContext(nc) as tc, ExitStack() as ctx:
        pin = ctx.enter_context(tc.tile_pool(name="pin", bufs=4))
        pmid = ctx.enter_context(tc.tile_pool(name="pmid", bufs=2))
        psm = ctx.enter_context(tc.tile_pool(name="psm", bufs=3))
        pout = ctx.enter_context(tc.tile_pool(name="pout", bufs=2))

        PF = 3  # input prefetch depth

        def issue_in(k):
            t = pin.tile([P, (F + 1) * FW], f32)
            src = bass.AP(
                xin,
                k * F * FW,
                [[(S + 1) * FW, BC], [SH * FW, H], [1, (F + 1) * FW]],
            )
            nc.gpsimd.dma_start(t[:], src)
            return t

        in_tiles = [issue_in(k) for k in range(PF)]

        for k in range(K):
            in_t = in_tiles[k]

            diff_t = pmid.tile([P, F * FW], f32)
            nc.vector.tensor_sub(
                diff_t[:], in_t[:, FW:(F + 1) * FW], in_t[:, 0:F * FW]
            )
            nc.scalar.activation(diff_t[:], diff_t[:], Af.Square)

            sq4 = diff_t[:].rearrange("p (f n c) -> p f n c", f=F, n=N, c=C)
            dist2_t = psm.tile([P, F * N], f32)
            d2 = dist2_t[:].rearrange("p (f n) -> p f n", f=F)
            nc.vector.tensor_add(d2, sq4[:, :, :, 0], sq4[:, :, :, 1])
            nc.vector.tensor_add(d2, d2, sq4[:, :, :, 2])
            # sqrt in place: dist2_t becomes dist
            nc.scalar.activation(dist2_t[:], dist2_t[:], Af.Sqrt)
            dist_t = dist2_t

            if k + PF < K:
                in_tiles.append(issue_in(k + PF))

            out_t = pout.tile([P, F * FW], f32)
            out4 = out_t[:].rearrange("p (f n c) -> p f n c", f=F, n=N, c=C)
            in4 = in_t[:, 0:F * FW].rearrange(
                "p (f n c) -> p f n c", f=F, n=N, c=C
            )
            dvb = (
                dist_t[:]
                .rearrange("p (f n) -> p f n", f=F)
                .unsqueeze(3)
                .broadcast_to([P, F, N, C])
            )
            nc.vector.tensor_add(out4, in4, dvb)

            dst = bass.AP(
                yout,
                k * F * FW,
                [[S * FW, BC], [SH * FW, H], [1, F * FW]],
            )
            nc.gpsimd.dma_start(dst, out_t[:])

    nc.compile()
    return nc


def kernel(x: np.ndarray, **_unused) -> np.ndarray:
    x = np.ascontiguousarray(np.asarray(x), dtype=np.float32)
    assert x.shape == (B, S, N, C), x.shape

    if "nc" not in _cache:
        _cache["nc"] = _build()
    nc = _cache["nc"]

    in_maps = []
    for ci in range(NCORES):
        xc = x[ci * BC:(ci + 1) * BC].reshape(BC, S * FW)
        xp = np.empty((BC, (S + 1) * FW), dtype=np.float32)
        xp[:, : S * FW] = xc
        # pad frame = copy of the last frame -> diff at s = S-1 is 0
        xp[:, S * FW:] = xc[:, (S - 1) * FW:]
        in_maps.append({"xin": xp.reshape(IN_FLAT)})

    res = run_bass_kernel_spmd(nc, in_maps, core_ids=list(range(NCORES)))
    _cache["last_results"] = res

    out = np.empty((B, S, N, C), dtype=np.float32)
    for ci in range(NCORES):
        out[ci * BC:(ci + 1) * BC] = res.results[ci]["yout"].reshape(
            BC, S, N, C
        )
    return out



# revision 4
# speedup vs baseline: 1.8855x; 1.8855x over previous
"""Trainium2 Bass kernel for nn_PositionalEncoding_61151744360729.

out[b, s, n, :] = x[b, s, n, :] + ||x[b, s+1, n, :] - x[b, s, n, :]||_2
(with distance 0 at s = S-1).

Sharding: data-parallel on batch across 8 NeuronCores (64 batches/core).

Device layout (prepared host-side): fp16, coordinate-plane separated and
node-padded -- xin[b, c, s, n'] with n' in [0, 26) (node 25 is a zero pad
so every frame span is an even element count -> all DVE operands are
4-byte aligned and contiguous, which keeps the fp16 tensor_tensor ops in
2x perf mode). Each batch carries one extra frame (copy of the last), so
the frame-(S-1) distance is exactly 0 with no special-casing.

Per chunk of F frames (partition p = batch*2 + seq-half):
  DVE  shifted subtract over all 3 planes in one op (plane-tail entries
       are garbage and never consumed)
  ACT  square in place
  DVE  two adds fold the 3 planes into dist^2
  ACT  sqrt in place
  DVE  one broadcast add produces all 3 output planes
  DMA  out (SWDGE) while input loads ride HWDGE on the idle SP engine.

fp16 end-to-end halves HBM traffic vs f32; rel l2 error ~5e-4 against
the f32 reference, well inside the 2e-2 gate.
"""

import sys
from contextlib import ExitStack

for _p in ("/opt/trn_rl_repo", "/root/.axon_site/_ro/trn_rl_repo"):
    if _p not in sys.path:
        sys.path.insert(0, _p)

import numpy as np

import concourse.bass as bass
import concourse.tile as tile
from concourse import bacc, mybir
from concourse.bass_utils import run_bass_kernel_spmd

B, S, N, C = 512, 1024, 25, 3
W = 26                     # nodes padded to even count (fp16 4B alignment)
NCORES = 8
BC = B // NCORES           # 64 batches per core
H = 2                      # sequence halves -> 128 partitions
SH = S // H                # 512 frames per half
P = H * BC                 # 128 partitions
F = 64                     # frames per chunk per partition
K = SH // F                # 8 chunks
ISPAN = (F + 1) * W        # 1690  per-plane input span per chunk
OSPAN = F * W              # 1664  per-plane output span per chunk
IN_FLAT = BC * C * (S + 1) * W
OUT_FLAT = BC * C * S * W

_cache = {}


def _build():
    f16 = mybir.dt.float16
    Af = mybir.ActivationFunctionType
    nc = bacc.Bacc(
        "TRN2", target_bir_lowering=False, debug=False, num_devices=NCORES
    )
    xin = nc.dram_tensor("xin", [IN_FLAT], f16, kind="ExternalInput")
    yout = nc.dram_tensor("yout", [OUT_FLAT], f16, kind="ExternalOutput")

    with tile.TileContext(nc) as tc, ExitStack() as ctx:
        pin = ctx.enter_context(tc.tile_pool(name="pin", bufs=4))
        pmid = ctx.enter_context(tc.tile_pool(name="pmid", bufs=2))
        psm = ctx.enter_context(tc.tile_pool(name="psm", bufs=3))
        pout = ctx.enter_context(tc.tile_pool(name="pout", bufs=2))

        PF = 3  # input prefetch depth

        def issue_in(k):
            t = pin.tile([P, C * ISPAN], f16)
            for c in range(C):
                src = bass.AP(
                    xin,
                    c * (S + 1) * W + k * F * W,
                    [
                        [C * (S + 1) * W, BC],
                        [SH * W, H],
                        [1, ISPAN],
                    ],
                )
                nc.sync.dma_start(t[:, c * ISPAN:(c + 1) * ISPAN], src)
            return t

        in_tiles = [issue_in(k) for k in range(PF)]

        SUBL = C * ISPAN - W   # 5044: one shifted sub across all planes

        for k in range(K):
            in_t = in_tiles[k]

            diff_t = pmid.tile([P, C * ISPAN], f16)
            nc.vector.tensor_sub(
                diff_t[:, 0:SUBL], in_t[:, W:W + SUBL], in_t[:, 0:SUBL]
            )
            nc.scalar.activation(
                diff_t[:, 0:SUBL], diff_t[:, 0:SUBL], Af.Square
            )

            dist_t = psm.tile([P, OSPAN], f16)
            nc.vector.tensor_add(
                dist_t[:], diff_t[:, 0:OSPAN],
                diff_t[:, ISPAN:ISPAN + OSPAN],
            )
            nc.vector.tensor_add(
                dist_t[:], dist_t[:],
                diff_t[:, 2 * ISPAN:2 * ISPAN + OSPAN],
            )
            nc.scalar.activation(dist_t[:], dist_t[:], Af.Sqrt)

            if k + PF < K:
                in_tiles.append(issue_in(k + PF))

            out_t = pout.tile([P, C * OSPAN], f16)
            out3 = out_t[:].rearrange("p (c l) -> p c l", c=C)
            in3 = in_t[:].rearrange("p (c l) -> p c l", c=C)[:, :, 0:OSPAN]
            db = dist_t[:].unsqueeze(1).broadcast_to([P, C, OSPAN])
            nc.vector.tensor_add(out3, in3, db)

            for c in range(C):
                dst = bass.AP(
                    yout,
                    c * S * W + k * F * W,
                    [
                        [C * S * W, BC],
                        [SH * W, H],
                        [1, OSPAN],
                    ],
                )
                nc.gpsimd.dma_start(
                    dst, out_t[:, c * OSPAN:(c + 1) * OSPAN]
                )

    nc.compile()
    return nc


def kernel(x: np.ndarray, **_unused) -> np.ndarray:
    x = np.asarray(x)
    assert x.shape == (B, S, N, C), x.shape

    if "nc" not in _cache:
        _cache["nc"] = _build()
    nc = _cache["nc"]

    # [B, S, N, C] f32 -> [B, C, S, N] fp16, node-padded to W, one extra
    # frame per batch (copy of the last -> distance 0 at s = S-1).
    xt = x.astype(np.float16).transpose(0, 3, 1, 2)  # [B, C, S, N]
    in_maps = []
    for ci in range(NCORES):
        xp = np.zeros((BC, C, S + 1, W), dtype=np.float16)
        xc = xt[ci * BC:(ci + 1) * BC]
        xp[:, :, :S, :N] = xc
        xp[:, :, S, :N] = xc[:, :, S - 1, :]
        in_maps.append({"xin": xp.reshape(IN_FLAT)})

    res = run_bass_kernel_spmd(nc, in_maps, core_ids=list(range(NCORES)))
    _cache["last_results"] = res

    out = np.empty((B, S, N, C), dtype=np.float32)
    for ci in range(NCORES):
        yo = res.results[ci]["yout"].reshape(BC, C, S, W)
        out[ci * BC:(ci + 1) * BC] = (
            yo[:, :, :, :N].transpose(0, 2, 3, 1).astype(np.float32)
        )
    return out
